# revision 19
# baseline (speedup 1.0000x reference)
"""Trainium2 Bass kernel for nn_AtenMmQuint8: quint8 dense matmul.

    out = ((x - 65) * 0.199) @ ((y - 160) * 0.0215)
    x: [2048, 4096] int32 (quint8 values 0..255)
    y: [4096, 2048] int32 (quint8 values 0..255)
    out: [2048, 2048] fp32

Strategy (v2, fp8 DoubleRow): the correctness gate is rel_err < 2e-2 and
the output is dominated by a large common term (all entries ~ -35.6k +- 2k
in dequant units), so the integer-domain error budget per element is
~100k+ units.  Quantizing both operands to fp8 e4m3 (round-to-nearest)
keeps the total matmul error well inside that budget, which unlocks the
PE's fp8 DoubleRow mode: 256 contraction rows per matmul instead of 128,
i.e. half the bf16 matmul count.

Numerics (verified against the exact int reference on the real inputs):
  - x is re-centered on the host: xc = x - 127 in [-127, 128], so its
    fp8 rounding error (rms 1.79) is much smaller than for x-65 up to
    190 (rms 2.68).  The zero-point shift is corrected EXACTLY:
      out = (xc + 62) @ yd = xc@yd + 62 * colsum(yd)[n]
    The per-n correction is folded into the PSUM->SBUF copy as a
    per-partition bias (the device computes out.T, so n is the
    partition dim).  colsum(yd) is computed exactly on the host in
    int64 (it is part of the affine identity, not an approximation).
  - y ships as fp8(y - 160) directly (rms 2.28).
  - Measured end-to-end: relmax 9.4e-3 vs the 2e-2 gate.

Sharding: 4x2 tensor-parallel grid (4 M-blocks x 2 N-blocks); per-core
block out.T[1024 n, 512 m] = (x_block @ y_block).T.  The device computes
the TRANSPOSED block: stationary operand = y k-tile slice [128k x 128n]
(so out partitions = n and the zero-point bias is per-partition),
moving operand = xT k-tile slice [128k x 512m].

Device kernel (identical SPMD program on all 8 cores):
  - K interleaved across SBUF partitions (k = p*32 + j) exactly as in
    the bf16 kernel; a DoubleRow matmul contracts the (j=2J, j=2J+1)
    pair of k-tiles in one instruction: lhsT/rhs APs are [128, 2, f]
    with the middle dim selecting the pair (sim/ISA-verified layout).
  - 16 double-k-tiles x 8 n-blocks = 128 matmuls; MEASURED warm issue
    rate ~216ns/MM (same N=512 streaming rate as bf16, 2x the MACs)
    -> 27.6us PE floor vs 55.3us for the 256-MM bf16 kernel.
  - The binding resource after that is HBM supply: the three DMA
    queues (2x HWDGE + gpsimd SWDGE) share ~250-280 B/ns aggregate
    while the stream demands 222 B/ns, so the schedule (see comment
    at the scheds) is everything and a few 0.3-2us jitter stalls
    remain.
  - PE prewarm as in v1: throwaway matmuls bridge the gap from the
    framework entry barrier to first-data so the HAM clock ramp is
    done before the real stream starts.  Early PE-idle gaps must stay
    well under ~1us or the un-throttle slips and the early stream
    runs at 1.2GHz (427ns/MM) -- measured, costs multiple us.
  - PSUM: bank nb accumulates n-block nb over all 16 double tiles;
    the last TAIL_J double tiles run nb-major so banks retire one at
    a time: VectorE does (psum * SCALE + bias[n]) in one tensor_scalar
    pass; each 256KB store is split in column halves across the two
    HWDGE rings (loads are SWDGE-only by then), and the final bank
    also splits its scale pass so the kernel-ending chain is
    TS(376) -> 128KB store overlapping the second half.

Measured on the target: 72.6us (bf16 v1 baseline) -> ~47.5us
  = 7.2 entry preamble + ~3.7 warmup/first-data + ~29.5 stream
    (27.6 floor + jitter) + ~1.9 retire/store drain + ~3.7 exit
    (entry/exit are framework-fixed; rel err 9.4e-3 vs 2e-2 gate).
"""

import numpy as np

import concourse.bass as bass  # noqa: F401  (kept for callers/debugging)
import concourse.mybir as mybir
import concourse.tile as tile
from concourse import bacc
from concourse.bass_utils import run_bass_kernel_spmd

X_ZP, Y_ZP = 65, 160
X_SHIFT = 127                # host recenter for x; see _stage()
SCALE = 0.199 * 0.0215

M, K, N = 2048, 4096, 2048
GM, GN = 4, 2                # core grid: 4 M-blocks x 2 N-blocks
MC, NC = M // GM, N // GN    # 512 x 1024 per-core output block
P = 128
KT = K // P                  # 32 k-tiles
KJ = KT // 2                 # 16 double k-tiles (DoubleRow)
NBLK = NC // P               # 8 n-blocks == 8 PSUM banks
NB = 512                     # psum bank free size / matmul moving free dim
TAIL_J = 5                   # trailing double-tiles run nb-major (retire)
N_WARM = 30                  # bridge entry-barrier -> first data (~3.2us)

# Load schedules (one FIFO list per DMA ring).  Only SP ("sync") and
# Activation ("scalar") have HWDGE rings; gpsimd DMAs via SWDGE
# (measured 170 B/ns sustained, zero idle).  Pieces:
#   ('x', a, b)  : xb[:, a:b, :]    <- xTr[:, a:b, :]   ((b-a) * 64KB)
#   ('y', a, b)  : yb[:, a:b, :]    <- ysr[:, a:b, :]   ((b-a) * 128KB)
#   ('yh', j, h) : yb[:, j, 512h:512h+512] column half  (64KB)
#   ('bias',)    : bias_sb <- bias dram                 (4KB)
# The tile framework's cross-engine dependencies are byte-range
# bounding boxes, so every matmul of a j-pair gates on ALL FOUR column
# halves of that pair's y (not just its own nb block) -- measured the
# hard way: a schedule that deferred the h1 halves stalled the early
# stream ~1.7us and pushed the HAM un-throttle from 12.5us to 18.3us
# (early PE gaps >~1us delay the clock ramp; <=400ns gaps do not).
# The three queues share ~250-280 B/ns of HBM read bandwidth while the
# stream is running (measured; a queue added does NOT add aggregate),
# so the layout keeps delivery in exact consumption order: the j0-j3
# halves lead both HWDGE rings (h-pairs split across rings so the two
# T0-critical h0 pieces land in parallel), y then alternates ring by
# parity (singles j4-7, pairs j8-27), while SWDGE -- measured at a
# consistent 128-173 B/ns with zero idle -- carries ALL of x in
# deadline order plus the last y pairs, leaving the HWDGE rings idle
# when the tail's output stores need them.
SYNC_SCHED = (
    ("yh", 0, 0), ("yh", 0, 1), ("yh", 2, 0), ("yh", 2, 1),
    ("y", 4, 5), ("y", 6, 7), ("y", 8, 10), ("y", 12, 14),
    ("y", 16, 18), ("y", 20, 22), ("y", 24, 26),
)
SCALAR_SCHED = (
    ("yh", 1, 0), ("yh", 1, 1), ("yh", 3, 0), ("yh", 3, 1),
    ("y", 5, 6), ("y", 7, 8), ("y", 10, 12), ("y", 14, 16),
    ("y", 18, 20), ("y", 22, 24), ("y", 26, 28),
)
# the latest pieces ride SWDGE (idle from ~20us) so the HWDGE rings are
# clear for the tail's output stores.
GPSIMD_SCHED = (
    ("x", 0, 2), ("x", 2, 4), ("x", 4, 6), ("x", 6, 8), ("x", 8, 10),
    ("x", 10, 12), ("x", 12, 14), ("x", 14, 16), ("x", 16, 20),
    ("x", 20, 24), ("x", 24, 28), ("x", 28, 32), ("y", 28, 30),
    ("y", 30, 32), ("bias",),
)


def _check_cover(scheds):
    cover = {("x", j): 0.0 for j in range(KT)}
    cover.update({("y", j): 0.0 for j in range(KT)})
    nbias = 0
    for sched in scheds:
        for item in sched:
            if item[0] == "yh":
                cover[("y", item[1])] += 0.5
            elif item[0] == "bias":
                nbias += 1
            else:
                for j in range(item[1], item[2]):
                    cover[(item[0], j)] += 1
    assert all(v == 1 for v in cover.values()), cover
    assert nbias == 1


_check_cover((SYNC_SCHED, SCALAR_SCHED, GPSIMD_SCHED))


def _emit(tc, xT, ys, bias, outT, tail_j=TAIL_J, n_warm=N_WARM):
    """Emit the per-core device program.

    xT:   [4096, 512]  fp8 DRAM (x block, K-major, host-recentered)
    ys:   [4096, 1024] fp8 DRAM (y block, zero-point subtracted)
    bias: [128, 8]     fp32 DRAM (bias[p, nb] for psum bank nb)
    outT: [1024, 512]  fp32 DRAM (transposed output block)
    """
    nc = tc.nc
    fp32 = mybir.dt.float32
    fp8 = mybir.dt.float8e4

    with (
        tc.tile_pool(name="sb", bufs=1) as sbp,
        tc.tile_pool(name="osb", bufs=NBLK, space="SBUF") as osbp,
        tc.tile_pool(name="ps", bufs=NBLK, space="PSUM") as psp,
    ):
        xb = sbp.tile([P, KT, MC], fp8, name="xb")
        yb = sbp.tile([P, KT, NC], fp8, name="yb")
        bias_sb = sbp.tile([P, NBLK], fp32, name="bias_sb")
        wt = sbp.tile([P, P], fp8, name="wt")
        psum = [psp.tile([P, NB], fp32, tag="ps", name=f"ps_{n}") for n in range(NBLK)]

        # K interleaved across partitions (k = p*KT + j): each partition's
        # j-range is one contiguous DRAM run.
        xTr = xT.rearrange("(p j) m -> p j m", j=KT)
        ysr = ys.rearrange("(p j) n -> p j n", j=KT)

        def issue(eng, sched):
            for item in sched:
                if item[0] == "yh":
                    j, h = item[1], item[2]
                    cs = slice(h * NB, (h + 1) * NB)
                    eng.dma_start(yb[:, j, cs], ysr[:, j, cs])
                elif item[0] == "x":
                    a, b = item[1], item[2]
                    eng.dma_start(xb[:, a:b, :], xTr[:, a:b, :])
                elif item[0] == "y":
                    a, b = item[1], item[2]
                    eng.dma_start(yb[:, a:b, :], ysr[:, a:b, :])
                else:
                    eng.dma_start(bias_sb[:], bias[:])

        nc.vector.memset(wt[:], 0.0)
        issue(nc.sync, SYNC_SCHED)
        issue(nc.scalar, SCALAR_SCHED)
        issue(nc.gpsimd, GPSIMD_SCHED)

        # HAM prewarm: keep the PE continuously busy from the framework
        # entry barrier until the first data lands (see v1 docstring).
        for _ in range(n_warm):
            nc.tensor.matmul(psum[0][:, :P], wt[:], wt[:], start=True, stop=True)

        def mm(J, nb):
            nc.tensor.matmul(
                psum[nb][:],
                yb[:, 2 * J : 2 * J + 2, nb * P : (nb + 1) * P],
                xb[:, 2 * J : 2 * J + 2, :],
                start=(J == 0),
                stop=(J == KJ - 1),
                perf_mode=mybir.MatmulPerfMode.DoubleRow,
            )

        # J-outer: touch every psum bank each double tile so the PE stream
        # stays dense while loads race ahead.
        for J in range(KJ - tail_j):
            for nb in range(NBLK):
                mm(J, nb)
        # nb-major tail: each PSUM bank finishes its K accumulation alone,
        # so its scale+bias copy and 256KB store overlap the remaining
        # matmuls of the other banks (stores alternate rings: 2MB total
        # needs both).
        for nb in range(NBLK):
            for J in range(KJ - tail_j, KJ):
                mm(J, nb)
            osb = osbp.tile([P, NB], fp32, tag="osb", name=f"osb_{nb}")
            rows = slice(nb * P, (nb + 1) * P)
            h = NB // 2
            if nb < NBLK - 1:
                nc.vector.tensor_scalar(
                    osb[:], psum[nb][:], SCALE, bias_sb[:, nb : nb + 1],
                    mybir.AluOpType.mult, mybir.AluOpType.add,
                )
                # store in column halves on both rings: halves each
                # store's drain (the load schedules are long done by the
                # time stores begin).
                nc.sync.dma_start(outT[rows, 0:h], osb[:, 0:h])
                nc.scalar.dma_start(outT[rows, h:NB], osb[:, h:NB])
            else:
                # last bank is the kernel-ending chain: retire in column
                # halves so the first store overlaps the second scale
                # pass, shortening the post-last-matmul drain.
                nc.vector.tensor_scalar(
                    osb[:, 0:h], psum[nb][:, 0:h], SCALE,
                    bias_sb[:, nb : nb + 1],
                    mybir.AluOpType.mult, mybir.AluOpType.add,
                )
                nc.sync.dma_start(outT[rows, 0:h], osb[:, 0:h])
                nc.vector.tensor_scalar(
                    osb[:, h:NB], psum[nb][:, h:NB], SCALE,
                    bias_sb[:, nb : nb + 1],
                    mybir.AluOpType.mult, mybir.AluOpType.add,
                )
                nc.scalar.dma_start(outT[rows, h:NB], osb[:, h:NB])


def _build_nc(**emit_kw):
    nc = bacc.Bacc("TRN2", target_bir_lowering=False, debug=False)
    fp8 = mybir.dt.float8e4
    xT = nc.declare_dram_parameter("xT", [K, MC], fp8, isOutput=False)
    ys = nc.declare_dram_parameter("ys", [K, NC], fp8, isOutput=False)
    bias = nc.declare_dram_parameter("bias", [P, NBLK], mybir.dt.float32,
                                     isOutput=False)
    outT = nc.declare_dram_parameter("outT", [NC, MC], mybir.dt.float32,
                                     isOutput=True)
    with tile.TileContext(nc) as tc:
        _emit(tc, xT[:], ys[:], bias[:], outT[:], **emit_kw)
    nc.compile()
    return nc


_CACHE = {}


def _get_nc():
    if "nc" not in _CACHE:
        _CACHE["nc"] = _build_nc()
    return _CACHE["nc"]


def _stage(x, y):
    """Host staging: fp8 operands + exact zero-point-shift bias."""
    fp8_np = mybir.dt.np(mybir.dt.float8e4)
    # x recentered to [-127, 128]; correction is exact via colsum(yd).
    xc8 = (x.astype(np.float32) - np.float32(X_SHIFT)).astype(fp8_np)
    yd8 = (y.astype(np.float32) - np.float32(Y_ZP)).astype(fp8_np)
    xT8 = np.ascontiguousarray(xc8.T)  # [K, M] fp8
    # out = xc@yd + (X_SHIFT - X_ZP) * colsum(yd);  62 = 127 - 65
    colsum = (y.astype(np.int64) - Y_ZP).sum(axis=0)  # [N] exact
    biasvec = (float(X_SHIFT - X_ZP) * colsum.astype(np.float64) * SCALE
               ).astype(np.float32)
    return xT8, yd8, biasvec


def kernel(x, y):
    x = np.asarray(x)
    y = np.asarray(y)
    assert x.shape == (M, K) and y.shape == (K, N)
    xT8, yd8, biasvec = _stage(x, y)

    in_maps = []
    for i in range(GM * GN):
        mi, ni = divmod(i, GN)
        bv = biasvec[ni * NC : (ni + 1) * NC].reshape(NBLK, P).T  # [P, NBLK]
        in_maps.append(
            {
                "xT": np.ascontiguousarray(xT8[:, mi * MC : (mi + 1) * MC]),
                "ys": np.ascontiguousarray(yd8[:, ni * NC : (ni + 1) * NC]),
                "bias": np.ascontiguousarray(bv),
            }
        )

    res = run_bass_kernel_spmd(_get_nc(), in_maps, list(range(GM * GN)))
    _CACHE["last_results"] = res

    out = np.empty((M, N), np.float32)
    for i in range(GM * GN):
        mi, ni = divmod(i, GN)
        out[mi * MC : (mi + 1) * MC, ni * NC : (ni + 1) * NC] = (
            res.results[i]["outT"].T
        )
    return out


# revision 22
# speedup vs baseline: 1.0105x; 1.0105x over previous
"""Trainium2 Bass kernel for nn_AtenMmQuint8: quint8 dense matmul.

    out = ((x - 65) * 0.199) @ ((y - 160) * 0.0215)
    x: [2048, 4096] int32 (quint8 values 0..255)
    y: [4096, 2048] int32 (quint8 values 0..255)
    out: [2048, 2048] fp32

Strategy (v2, fp8 DoubleRow): the correctness gate is rel_err < 2e-2 and
the output is dominated by a large common term (all entries ~ -35.6k +- 2k
in dequant units), so the integer-domain error budget per element is
~100k+ units.  Quantizing both operands to fp8 e4m3 (round-to-nearest)
keeps the total matmul error well inside that budget, which unlocks the
PE's fp8 DoubleRow mode: 256 contraction rows per matmul instead of 128,
i.e. half the bf16 matmul count.

Numerics (verified against the exact int reference on the real inputs):
  - x is re-centered on the host: xc = x - 127 in [-127, 128], so its
    fp8 rounding error (rms 1.79) is much smaller than for x-65 up to
    190 (rms 2.68).  The zero-point shift is corrected EXACTLY:
      out = (xc + 62) @ yd = xc@yd + 62 * colsum(yd)[n]
    The per-n correction is folded into the PSUM->SBUF copy as a
    per-partition bias (the device computes out.T, so n is the
    partition dim).  colsum(yd) is computed exactly on the host in
    int64 (it is part of the affine identity, not an approximation).
  - y ships as fp8(y - 160) directly (rms 2.28).
  - Measured end-to-end: relmax 9.4e-3 vs the 2e-2 gate.

Sharding: 4x2 tensor-parallel grid (4 M-blocks x 2 N-blocks); per-core
block out.T[1024 n, 512 m] = (x_block @ y_block).T.  The device computes
the TRANSPOSED block: stationary operand = y k-tile slice [128k x 128n]
(so out partitions = n and the zero-point bias is per-partition),
moving operand = xT k-tile slice [128k x 512m].

Device kernel (identical SPMD program on all 8 cores):
  - K interleaved across SBUF partitions (k = p*32 + j) exactly as in
    the bf16 kernel; a DoubleRow matmul contracts the (j=2J, j=2J+1)
    pair of k-tiles in one instruction: lhsT/rhs APs are [128, 2, f]
    with the middle dim selecting the pair (sim/ISA-verified layout).
  - 16 double-k-tiles x 8 n-blocks = 128 matmuls; MEASURED warm issue
    rate ~216ns/MM (same N=512 streaming rate as bf16, 2x the MACs)
    -> 27.6us PE floor vs 55.3us for the 256-MM bf16 kernel.
  - The binding resource after that is HBM supply: the three DMA
    queues (2x HWDGE + gpsimd SWDGE) share ~250-280 B/ns aggregate
    while the stream demands 222 B/ns, so the schedule (see comment
    at the scheds) is everything and a few 0.3-2us jitter stalls
    remain.
  - PE prewarm as in v1: throwaway matmuls bridge the gap from the
    framework entry barrier to first-data so the HAM clock ramp is
    done before the real stream starts.  Early PE-idle gaps must stay
    well under ~1us or the un-throttle slips and the early stream
    runs at 1.2GHz (427ns/MM) -- measured, costs multiple us.
  - PSUM: bank nb accumulates n-block nb over all 16 double tiles;
    the last TAIL_J double tiles run nb-major so banks retire one at
    a time: VectorE does (psum * SCALE + bias[n]) in one tensor_scalar
    pass; each 256KB store is split in column halves across the two
    HWDGE rings (loads are SWDGE-only by then), and the final bank
    also splits its scale pass so the kernel-ending chain is
    TS(376) -> 128KB store overlapping the second half.

Measured on the target: 72.6us (bf16 v1 baseline) -> ~47.5us
  = 7.2 entry preamble + ~3.7 warmup/first-data + ~29.5 stream
    (27.6 floor + jitter) + ~1.9 retire/store drain + ~3.7 exit
    (entry/exit are framework-fixed; rel err 9.4e-3 vs 2e-2 gate).
"""

import numpy as np

import concourse.bass as bass  # noqa: F401  (kept for callers/debugging)
import concourse.mybir as mybir
import concourse.tile as tile
from concourse import bacc
from concourse.bass_utils import run_bass_kernel_spmd

X_ZP, Y_ZP = 65, 160
X_SHIFT = 127                # host recenter for x; see _stage()
SCALE = 0.199 * 0.0215

M, K, N = 2048, 4096, 2048
GM, GN = 4, 2                # core grid: 4 M-blocks x 2 N-blocks
MC, NC = M // GM, N // GN    # 512 x 1024 per-core output block
P = 128
KT = K // P                  # 32 k-tiles
KJ = KT // 2                 # 16 double k-tiles (DoubleRow)
NBLK = NC // P               # 8 n-blocks == 8 PSUM banks
NB = 512                     # psum bank free size / matmul moving free dim
TAIL_J = 5                   # trailing double-tiles run nb-major (retire)
N_WARM = 27                  # N=128 warmups after the const-AP burst
N_WARM0 = 10                 # dependency-free N=1 warmups (const-AP weights)

# Load schedules (one FIFO list per DMA ring).  Only SP ("sync") and
# Activation ("scalar") have HWDGE rings; gpsimd DMAs via SWDGE
# (measured 170 B/ns sustained, zero idle).  Pieces:
#   ('x', a, b)  : xb[:, a:b, :]    <- xTr[:, a:b, :]   ((b-a) * 64KB)
#   ('y', a, b)  : yb[:, a:b, :]    <- ysr[:, a:b, :]   ((b-a) * 128KB)
#   ('yh', j, h) : yb[:, j, 512h:512h+512] column half  (64KB)
#   ('bias',)    : bias_sb <- bias dram                 (4KB)
# The tile framework's cross-engine dependencies are byte-range
# bounding boxes, so every matmul of a j-pair gates on ALL FOUR column
# halves of that pair's y (not just its own nb block) -- measured the
# hard way: a schedule that deferred the h1 halves stalled the early
# stream ~1.7us and pushed the HAM un-throttle from 12.5us to 18.3us
# (early PE gaps >~1us delay the clock ramp; <=400ns gaps do not).
# The three queues share ~250-280 B/ns of HBM read bandwidth while the
# stream is running (measured; a queue added does NOT add aggregate),
# so the layout keeps delivery in exact consumption order: the j0-j3
# halves lead both HWDGE rings (h-pairs split across rings so the two
# T0-critical h0 pieces land in parallel), y then alternates ring by
# parity (singles j4-7, pairs j8-27), while SWDGE -- measured at a
# consistent 128-173 B/ns with zero idle -- carries ALL of x in
# deadline order plus the last y pairs, leaving the HWDGE rings idle
# when the tail's output stores need them.
SYNC_SCHED = (
    ("yh", 0, 0), ("yh", 0, 1), ("yh", 2, 0), ("yh", 2, 1),
    ("y", 4, 5), ("y", 6, 7), ("y", 8, 10), ("y", 12, 14),
    ("y", 16, 18), ("y", 20, 22), ("y", 24, 26),
)
SCALAR_SCHED = (
    ("yh", 1, 0), ("yh", 1, 1), ("yh", 3, 0), ("yh", 3, 1),
    ("y", 5, 6), ("y", 7, 8), ("y", 10, 12), ("y", 14, 16),
    ("y", 18, 20), ("y", 22, 24), ("y", 26, 28),
)
# the latest pieces ride SWDGE (idle from ~20us) so the HWDGE rings are
# clear for the tail's output stores.
GPSIMD_SCHED = (
    ("x", 0, 2), ("x", 2, 4), ("x", 4, 6), ("x", 6, 8), ("x", 8, 10),
    ("x", 10, 12), ("x", 12, 14), ("x", 14, 16), ("x", 16, 20),
    ("x", 20, 24), ("x", 24, 28), ("x", 28, 32), ("y", 28, 30),
    ("y", 30, 32), ("bias",),
)


def _check_cover(scheds):
    cover = {("x", j): 0.0 for j in range(KT)}
    cover.update({("y", j): 0.0 for j in range(KT)})
    nbias = 0
    for sched in scheds:
        for item in sched:
            if item[0] == "yh":
                cover[("y", item[1])] += 0.5
            elif item[0] == "bias":
                nbias += 1
            else:
                for j in range(item[1], item[2]):
                    cover[(item[0], j)] += 1
    assert all(v == 1 for v in cover.values()), cover
    assert nbias == 1


_check_cover((SYNC_SCHED, SCALAR_SCHED, GPSIMD_SCHED))


def _emit(tc, xT, ys, bias, outT, tail_j=TAIL_J, n_warm=N_WARM):
    """Emit the per-core device program.

    xT:   [4096, 512]  fp8 DRAM (x block, K-major, host-recentered)
    ys:   [4096, 1024] fp8 DRAM (y block, zero-point subtracted)
    bias: [128, 8]     fp32 DRAM (bias[p, nb] for psum bank nb)
    outT: [1024, 512]  fp32 DRAM (transposed output block)
    """
    nc = tc.nc
    fp32 = mybir.dt.float32
    fp8 = mybir.dt.float8e4

    with (
        tc.tile_pool(name="sb", bufs=1) as sbp,
        tc.tile_pool(name="osb", bufs=NBLK, space="SBUF") as osbp,
        tc.tile_pool(name="ps", bufs=NBLK, space="PSUM") as psp,
    ):
        xb = sbp.tile([P, KT, MC], fp8, name="xb")
        yb = sbp.tile([P, KT, NC], fp8, name="yb")
        bias_sb = sbp.tile([P, NBLK], fp32, name="bias_sb")
        wt = sbp.tile([P, P], fp8, name="wt")
        psum = [psp.tile([P, NB], fp32, tag="ps", name=f"ps_{n}") for n in range(NBLK)]

        # K interleaved across partitions (k = p*KT + j): each partition's
        # j-range is one contiguous DRAM run.
        xTr = xT.rearrange("(p j) m -> p j m", j=KT)
        ysr = ys.rearrange("(p j) n -> p j n", j=KT)

        def issue(eng, sched):
            for item in sched:
                if item[0] == "yh":
                    j, h = item[1], item[2]
                    cs = slice(h * NB, (h + 1) * NB)
                    eng.dma_start(yb[:, j, cs], ysr[:, j, cs])
                elif item[0] == "x":
                    a, b = item[1], item[2]
                    eng.dma_start(xb[:, a:b, :], xTr[:, a:b, :])
                elif item[0] == "y":
                    a, b = item[1], item[2]
                    eng.dma_start(yb[:, a:b, :], ysr[:, a:b, :])
                else:
                    eng.dma_start(bias_sb[:], bias[:])

        nc.vector.memset(wt[:], 0.0)
        issue(nc.sync, SYNC_SCHED)
        issue(nc.scalar, SCALAR_SCHED)
        issue(nc.gpsimd, GPSIMD_SCHED)

        # HAM prewarm: keep the PE continuously busy from the framework
        # entry barrier until the first data lands (see v1 docstring).
        # The first burst uses the framework's const-AP tile ([128,1]
        # fp32, memset by gpsimd BEFORE the entry barrier) as weights, so
        # the PE starts the moment its own preamble ends instead of
        # waiting ~0.5us for the DVE memset of wt to land.
        zap = nc.const_aps.aps[(mybir.dt.float32, 0.0)]
        for _ in range(N_WARM0):
            nc.tensor.matmul(psum[0][:1, :1], zap, zap, start=True, stop=True)
        for _ in range(n_warm):
            nc.tensor.matmul(psum[0][:, :P], wt[:], wt[:], start=True, stop=True)

        def mm(J, nb):
            nc.tensor.matmul(
                psum[nb][:],
                yb[:, 2 * J : 2 * J + 2, nb * P : (nb + 1) * P],
                xb[:, 2 * J : 2 * J + 2, :],
                start=(J == 0),
                stop=(J == KJ - 1),
                perf_mode=mybir.MatmulPerfMode.DoubleRow,
            )

        # J-outer: touch every psum bank each double tile so the PE stream
        # stays dense while loads race ahead.
        for J in range(KJ - tail_j):
            for nb in range(NBLK):
                mm(J, nb)
        # nb-major tail: each PSUM bank finishes its K accumulation alone,
        # so its scale+bias copy and 256KB store overlap the remaining
        # matmuls of the other banks (stores alternate rings: 2MB total
        # needs both).
        for nb in range(NBLK):
            for J in range(KJ - tail_j, KJ):
                mm(J, nb)
            osb = osbp.tile([P, NB], fp32, tag="osb", name=f"osb_{nb}")
            rows = slice(nb * P, (nb + 1) * P)
            h = NB // 2
            if nb < NBLK - 1:
                nc.vector.tensor_scalar(
                    osb[:], psum[nb][:], SCALE, bias_sb[:, nb : nb + 1],
                    mybir.AluOpType.mult, mybir.AluOpType.add,
                )
                # store in column halves on both rings: halves each
                # store's drain (the load schedules are long done by the
                # time stores begin).
                nc.sync.dma_start(outT[rows, 0:h], osb[:, 0:h])
                nc.scalar.dma_start(outT[rows, h:NB], osb[:, h:NB])
            else:
                # last bank is the kernel-ending chain: retire the two
                # column halves on TWO engines in parallel (DVE
                # tensor_scalar + ACT activation compute the same
                # psum*SCALE + bias[n]), each feeding its own store ring,
                # so the drain is one half-pass plus one 128KB store.
                nc.vector.tensor_scalar(
                    osb[:, 0:h], psum[nb][:, 0:h], SCALE,
                    bias_sb[:, nb : nb + 1],
                    mybir.AluOpType.mult, mybir.AluOpType.add,
                )
                nc.sync.dma_start(outT[rows, 0:h], osb[:, 0:h])
                nc.scalar.activation(
                    osb[:, h:NB], psum[nb][:, h:NB],
                    mybir.ActivationFunctionType.Identity,
                    bias=bias_sb[:, nb : nb + 1], scale=float(SCALE),
                )
                nc.scalar.dma_start(outT[rows, h:NB], osb[:, h:NB])


def _build_nc(**emit_kw):
    nc = bacc.Bacc("TRN2", target_bir_lowering=False, debug=False)
    fp8 = mybir.dt.float8e4
    xT = nc.declare_dram_parameter("xT", [K, MC], fp8, isOutput=False)
    ys = nc.declare_dram_parameter("ys", [K, NC], fp8, isOutput=False)
    bias = nc.declare_dram_parameter("bias", [P, NBLK], mybir.dt.float32,
                                     isOutput=False)
    outT = nc.declare_dram_parameter("outT", [NC, MC], mybir.dt.float32,
                                     isOutput=True)
    with tile.TileContext(nc) as tc:
        _emit(tc, xT[:], ys[:], bias[:], outT[:], **emit_kw)
    nc.compile()
    return nc


_CACHE = {}


def _get_nc():
    if "nc" not in _CACHE:
        _CACHE["nc"] = _build_nc()
    return _CACHE["nc"]


def _stage(x, y):
    """Host staging: fp8 operands + exact zero-point-shift bias."""
    fp8_np = mybir.dt.np(mybir.dt.float8e4)
    # x recentered to [-127, 128]; correction is exact via colsum(yd).
    xc8 = (x.astype(np.float32) - np.float32(X_SHIFT)).astype(fp8_np)
    yd8 = (y.astype(np.float32) - np.float32(Y_ZP)).astype(fp8_np)
    xT8 = np.ascontiguousarray(xc8.T)  # [K, M] fp8
    # out = xc@yd + (X_SHIFT - X_ZP) * colsum(yd);  62 = 127 - 65
    colsum = (y.astype(np.int64) - Y_ZP).sum(axis=0)  # [N] exact
    biasvec = (float(X_SHIFT - X_ZP) * colsum.astype(np.float64) * SCALE
               ).astype(np.float32)
    return xT8, yd8, biasvec


def kernel(x, y):
    x = np.asarray(x)
    y = np.asarray(y)
    assert x.shape == (M, K) and y.shape == (K, N)
    xT8, yd8, biasvec = _stage(x, y)

    in_maps = []
    for i in range(GM * GN):
        mi, ni = divmod(i, GN)
        bv = biasvec[ni * NC : (ni + 1) * NC].reshape(NBLK, P).T  # [P, NBLK]
        in_maps.append(
            {
                "xT": np.ascontiguousarray(xT8[:, mi * MC : (mi + 1) * MC]),
                "ys": np.ascontiguousarray(yd8[:, ni * NC : (ni + 1) * NC]),
                "bias": np.ascontiguousarray(bv),
            }
        )

    res = run_bass_kernel_spmd(_get_nc(), in_maps, list(range(GM * GN)))
    _CACHE["last_results"] = res

    out = np.empty((M, N), np.float32)
    for i in range(GM * GN):
        mi, ni = divmod(i, GN)
        out[mi * MC : (mi + 1) * MC, ni * NC : (ni + 1) * NC] = (
            res.results[i]["outT"].T
        )
    return out


# revision 23
# speedup vs baseline: 1.0174x; 1.0068x over previous
"""Trainium2 Bass kernel for nn_AtenMmQuint8: quint8 dense matmul.

    out = ((x - 65) * 0.199) @ ((y - 160) * 0.0215)
    x: [2048, 4096] int32 (quint8 values 0..255)
    y: [4096, 2048] int32 (quint8 values 0..255)
    out: [2048, 2048] fp32

Strategy (v2, fp8 DoubleRow): the correctness gate is rel_err < 2e-2 and
the output is dominated by a large common term (all entries ~ -35.6k +- 2k
in dequant units), so the integer-domain error budget per element is
~100k+ units.  Quantizing both operands to fp8 e4m3 (round-to-nearest)
keeps the total matmul error well inside that budget, which unlocks the
PE's fp8 DoubleRow mode: 256 contraction rows per matmul instead of 128,
i.e. half the bf16 matmul count.

Numerics (verified against the exact int reference on the real inputs):
  - x is re-centered on the host: xc = x - 127 in [-127, 128], so its
    fp8 rounding error (rms 1.79) is much smaller than for x-65 up to
    190 (rms 2.68).  The zero-point shift is corrected EXACTLY:
      out = (xc + 62) @ yd = xc@yd + 62 * colsum(yd)[n]
    The per-n correction is folded into the PSUM->SBUF copy as a
    per-partition bias (the device computes out.T, so n is the
    partition dim).  colsum(yd) is computed exactly on the host in
    int64 (it is part of the affine identity, not an approximation).
  - y ships as fp8(y - 160) directly (rms 2.28).
  - Measured end-to-end: relmax 9.4e-3 vs the 2e-2 gate.

Sharding: 4x2 tensor-parallel grid (4 M-blocks x 2 N-blocks); per-core
block out.T[1024 n, 512 m] = (x_block @ y_block).T.  The device computes
the TRANSPOSED block: stationary operand = y k-tile slice [128k x 128n]
(so out partitions = n and the zero-point bias is per-partition),
moving operand = xT k-tile slice [128k x 512m].

Device kernel (identical SPMD program on all 8 cores):
  - K interleaved across SBUF partitions (k = p*32 + j) exactly as in
    the bf16 kernel; a DoubleRow matmul contracts the (j=2J, j=2J+1)
    pair of k-tiles in one instruction: lhsT/rhs APs are [128, 2, f]
    with the middle dim selecting the pair (sim/ISA-verified layout).
  - 16 double-k-tiles x 8 n-blocks = 128 matmuls; MEASURED warm issue
    rate ~216ns/MM (same N=512 streaming rate as bf16, 2x the MACs)
    -> 27.6us PE floor vs 55.3us for the 256-MM bf16 kernel.
  - The binding resource after that is HBM supply: the three DMA
    queues (2x HWDGE + gpsimd SWDGE) share ~250-280 B/ns aggregate
    while the stream demands 222 B/ns, so the schedule (see comment
    at the scheds) is everything and a few 0.3-2us jitter stalls
    remain.
  - PE prewarm as in v1: throwaway matmuls bridge the gap from the
    framework entry barrier to first-data so the HAM clock ramp is
    done before the real stream starts.  Early PE-idle gaps must stay
    well under ~1us or the un-throttle slips and the early stream
    runs at 1.2GHz (427ns/MM) -- measured, costs multiple us.
  - PSUM: bank nb accumulates n-block nb over all 16 double tiles;
    the last TAIL_J double tiles run nb-major so banks retire one at
    a time: VectorE does (psum * SCALE + bias[n]) in one tensor_scalar
    pass; each 256KB store is split in column halves across the two
    HWDGE rings (loads are SWDGE-only by then), and the final bank
    also splits its scale pass so the kernel-ending chain is
    TS(376) -> 128KB store overlapping the second half.

Measured on the target: 72.6us (bf16 v1 baseline) -> ~47.5us
  = 7.2 entry preamble + ~3.7 warmup/first-data + ~29.5 stream
    (27.6 floor + jitter) + ~1.9 retire/store drain + ~3.7 exit
    (entry/exit are framework-fixed; rel err 9.4e-3 vs 2e-2 gate).
"""

import numpy as np

import concourse.bass as bass  # noqa: F401  (kept for callers/debugging)
import concourse.mybir as mybir
import concourse.tile as tile
from concourse import bacc
from concourse.bass_utils import run_bass_kernel_spmd

X_ZP, Y_ZP = 65, 160
X_SHIFT = 127                # host recenter for x; see _stage()
SCALE = 0.199 * 0.0215

M, K, N = 2048, 4096, 2048
GM, GN = 4, 2                # core grid: 4 M-blocks x 2 N-blocks
MC, NC = M // GM, N // GN    # 512 x 1024 per-core output block
P = 128
KT = K // P                  # 32 k-tiles
KJ = KT // 2                 # 16 double k-tiles (DoubleRow)
NBLK = NC // P               # 8 n-blocks == 8 PSUM banks
NB = 512                     # psum bank free size / matmul moving free dim
TAIL_J = 5                   # trailing double-tiles run nb-major (retire)
N_WARM = 27                  # N=128 warmups after the const-AP burst
N_WARM0 = 10                 # dependency-free N=1 warmups (const-AP weights)

# Load schedules (one FIFO list per DMA ring).  Only SP ("sync") and
# Activation ("scalar") have HWDGE rings; gpsimd DMAs via SWDGE
# (measured 170 B/ns sustained, zero idle).  Pieces:
#   ('x', a, b)  : xb[:, a:b, :]    <- xTr[:, a:b, :]   ((b-a) * 64KB)
#   ('y', a, b)  : yb[:, a:b, :]    <- ysr[:, a:b, :]   ((b-a) * 128KB)
#   ('yh', j, h) : yb[:, j, 512h:512h+512] column half  (64KB)
#   ('bias',)    : bias_sb <- bias dram                 (4KB)
# The tile framework's cross-engine dependencies are byte-range
# bounding boxes, so every matmul of a j-pair gates on ALL FOUR column
# halves of that pair's y (not just its own nb block) -- measured the
# hard way: a schedule that deferred the h1 halves stalled the early
# stream ~1.7us and pushed the HAM un-throttle from 12.5us to 18.3us
# (early PE gaps >~1us delay the clock ramp; <=400ns gaps do not).
# The three queues share ~250-280 B/ns of HBM read bandwidth while the
# stream is running (measured; a queue added does NOT add aggregate),
# so the layout keeps delivery in exact consumption order: the j0-j3
# halves lead both HWDGE rings (h-pairs split across rings so the two
# T0-critical h0 pieces land in parallel), y then alternates ring by
# parity (singles j4-7, pairs j8-27), while SWDGE -- measured at a
# consistent 128-173 B/ns with zero idle -- carries ALL of x in
# deadline order plus the last y pairs, leaving the HWDGE rings idle
# when the tail's output stores need them.
SYNC_SCHED = (
    ("yh", 0, 0), ("yh", 0, 1), ("yh", 2, 0), ("yh", 2, 1),
    ("y", 4, 5), ("y", 6, 7), ("y", 8, 10), ("y", 12, 14),
    ("y", 16, 18), ("y", 20, 22), ("y", 24, 26),
)
SCALAR_SCHED = (
    ("yh", 1, 0), ("yh", 1, 1), ("yh", 3, 0), ("yh", 3, 1),
    ("y", 5, 6), ("y", 7, 8), ("y", 10, 12), ("y", 14, 16),
    ("y", 18, 20), ("y", 22, 24), ("y", 26, 28),
)
# the latest pieces ride SWDGE (idle from ~20us) so the HWDGE rings are
# clear for the tail's output stores.
GPSIMD_SCHED = (
    ("x", 0, 2), ("x", 2, 4), ("x", 4, 6), ("x", 6, 8), ("x", 8, 10),
    ("x", 10, 12), ("x", 12, 14), ("x", 14, 16), ("x", 16, 20),
    ("x", 20, 24), ("x", 24, 28), ("x", 28, 32), ("y", 28, 30),
    ("y", 30, 32), ("bias",),
)


def _check_cover(scheds):
    cover = {("x", j): 0.0 for j in range(KT)}
    cover.update({("y", j): 0.0 for j in range(KT)})
    nbias = 0
    for sched in scheds:
        for item in sched:
            if item[0] == "yh":
                cover[("y", item[1])] += 0.5
            elif item[0] == "bias":
                nbias += 1
            else:
                for j in range(item[1], item[2]):
                    cover[(item[0], j)] += 1
    assert all(v == 1 for v in cover.values()), cover
    assert nbias == 1


_check_cover((SYNC_SCHED, SCALAR_SCHED, GPSIMD_SCHED))


def _emit(tc, xT, ys, bias, outT, tail_j=TAIL_J, n_warm=N_WARM):
    """Emit the per-core device program.

    xT:   [4096, 512]  fp8 DRAM (x block, K-major, host-recentered)
    ys:   [4096, 1024] fp8 DRAM (y block, zero-point subtracted)
    bias: [128, 8]     fp32 DRAM (bias[p, nb] for psum bank nb)
    outT: [1024, 512]  fp32 DRAM (transposed output block)
    """
    nc = tc.nc
    fp32 = mybir.dt.float32
    fp8 = mybir.dt.float8e4

    with (
        tc.tile_pool(name="sb", bufs=1) as sbp,
        tc.tile_pool(name="osb", bufs=NBLK, space="SBUF") as osbp,
        tc.tile_pool(name="ps", bufs=NBLK, space="PSUM") as psp,
    ):
        xb = sbp.tile([P, KT, MC], fp8, name="xb")
        yb = sbp.tile([P, KT, NC], fp8, name="yb")
        bias_sb = sbp.tile([P, NBLK], fp32, name="bias_sb")
        wt = sbp.tile([P, P], fp8, name="wt")
        psum = [psp.tile([P, NB], fp32, tag="ps", name=f"ps_{n}") for n in range(NBLK)]

        # K interleaved across partitions (k = p*KT + j): each partition's
        # j-range is one contiguous DRAM run.
        xTr = xT.rearrange("(p j) m -> p j m", j=KT)
        ysr = ys.rearrange("(p j) n -> p j n", j=KT)

        def issue(eng, sched):
            for item in sched:
                if item[0] == "yh":
                    j, h = item[1], item[2]
                    cs = slice(h * NB, (h + 1) * NB)
                    eng.dma_start(yb[:, j, cs], ysr[:, j, cs])
                elif item[0] == "x":
                    a, b = item[1], item[2]
                    eng.dma_start(xb[:, a:b, :], xTr[:, a:b, :])
                elif item[0] == "y":
                    a, b = item[1], item[2]
                    eng.dma_start(yb[:, a:b, :], ysr[:, a:b, :])
                else:
                    eng.dma_start(bias_sb[:], bias[:])

        nc.vector.memset(wt[:], 0.0)
        issue(nc.sync, SYNC_SCHED)
        issue(nc.scalar, SCALAR_SCHED)
        issue(nc.gpsimd, GPSIMD_SCHED)

        # HAM prewarm: keep the PE continuously busy from the framework
        # entry barrier until the first data lands (see v1 docstring).
        # The first burst uses the framework's const-AP tile ([128,1]
        # fp32, memset by gpsimd BEFORE the entry barrier) as weights, so
        # the PE starts the moment its own preamble ends instead of
        # waiting ~0.5us for the DVE memset of wt to land.
        zap = nc.const_aps.aps[(mybir.dt.float32, 0.0)]
        for _ in range(N_WARM0):
            nc.tensor.matmul(psum[0][:1, :1], zap, zap, start=True, stop=True)
        for _ in range(n_warm):
            nc.tensor.matmul(psum[0][:, :P], wt[:], wt[:], start=True, stop=True)

        def mm(J, nb):
            nc.tensor.matmul(
                psum[nb][:],
                yb[:, 2 * J : 2 * J + 2, nb * P : (nb + 1) * P],
                xb[:, 2 * J : 2 * J + 2, :],
                start=(J == 0),
                stop=(J == KJ - 1),
                perf_mode=mybir.MatmulPerfMode.DoubleRow,
            )

        # J-outer: touch every psum bank each double tile so the PE stream
        # stays dense while loads race ahead.
        for J in range(KJ - tail_j):
            for nb in range(NBLK):
                mm(J, nb)
        # nb-major tail: each PSUM bank finishes its K accumulation alone,
        # so its scale+bias copy and 256KB store overlap the remaining
        # matmuls of the other banks (stores alternate rings: 2MB total
        # needs both).
        for nb in range(NBLK):
            for J in range(KJ - tail_j, KJ):
                mm(J, nb)
            osb = osbp.tile([P, NB], fp32, tag="osb", name=f"osb_{nb}")
            rows = slice(nb * P, (nb + 1) * P)
            h = NB // 2
            if nb < NBLK - 1:
                nc.vector.tensor_scalar(
                    osb[:], psum[nb][:], SCALE, bias_sb[:, nb : nb + 1],
                    mybir.AluOpType.mult, mybir.AluOpType.add,
                )
                # store in column halves on both rings: halves each
                # store's drain (the load schedules are long done by the
                # time stores begin).
                nc.sync.dma_start(outT[rows, 0:h], osb[:, 0:h])
                nc.scalar.dma_start(outT[rows, h:NB], osb[:, h:NB])
            else:
                # last bank is the kernel-ending chain: retire in column
                # QUARTERS, each store issued the moment its quarter's
                # scale pass lands, quarters alternating rings -- the
                # drain pipelines down to ~one TS pass + one 64KB store.
                # (An ACT-engine half was tried for engine-parallel
                # retirement: measured ~2x slower than DVE per element,
                # net wash.)
                q = NB // 4
                for k in range(4):
                    cs = slice(k * q, (k + 1) * q)
                    nc.vector.tensor_scalar(
                        osb[:, cs], psum[nb][:, cs], SCALE,
                        bias_sb[:, nb : nb + 1],
                        mybir.AluOpType.mult, mybir.AluOpType.add,
                    )
                    eng = nc.sync if k % 2 == 0 else nc.scalar
                    eng.dma_start(outT[rows, cs], osb[:, cs])


def _build_nc(**emit_kw):
    nc = bacc.Bacc("TRN2", target_bir_lowering=False, debug=False)
    fp8 = mybir.dt.float8e4
    xT = nc.declare_dram_parameter("xT", [K, MC], fp8, isOutput=False)
    ys = nc.declare_dram_parameter("ys", [K, NC], fp8, isOutput=False)
    bias = nc.declare_dram_parameter("bias", [P, NBLK], mybir.dt.float32,
                                     isOutput=False)
    outT = nc.declare_dram_parameter("outT", [NC, MC], mybir.dt.float32,
                                     isOutput=True)
    with tile.TileContext(nc) as tc:
        _emit(tc, xT[:], ys[:], bias[:], outT[:], **emit_kw)
    nc.compile()
    return nc


_CACHE = {}


def _get_nc():
    if "nc" not in _CACHE:
        _CACHE["nc"] = _build_nc()
    return _CACHE["nc"]


def _stage(x, y):
    """Host staging: fp8 operands + exact zero-point-shift bias."""
    fp8_np = mybir.dt.np(mybir.dt.float8e4)
    # x recentered to [-127, 128]; correction is exact via colsum(yd).
    xc8 = (x.astype(np.float32) - np.float32(X_SHIFT)).astype(fp8_np)
    yd8 = (y.astype(np.float32) - np.float32(Y_ZP)).astype(fp8_np)
    xT8 = np.ascontiguousarray(xc8.T)  # [K, M] fp8
    # out = xc@yd + (X_SHIFT - X_ZP) * colsum(yd);  62 = 127 - 65
    colsum = (y.astype(np.int64) - Y_ZP).sum(axis=0)  # [N] exact
    biasvec = (float(X_SHIFT - X_ZP) * colsum.astype(np.float64) * SCALE
               ).astype(np.float32)
    return xT8, yd8, biasvec


def kernel(x, y):
    x = np.asarray(x)
    y = np.asarray(y)
    assert x.shape == (M, K) and y.shape == (K, N)
    xT8, yd8, biasvec = _stage(x, y)

    in_maps = []
    for i in range(GM * GN):
        mi, ni = divmod(i, GN)
        bv = biasvec[ni * NC : (ni + 1) * NC].reshape(NBLK, P).T  # [P, NBLK]
        in_maps.append(
            {
                "xT": np.ascontiguousarray(xT8[:, mi * MC : (mi + 1) * MC]),
                "ys": np.ascontiguousarray(yd8[:, ni * NC : (ni + 1) * NC]),
                "bias": np.ascontiguousarray(bv),
            }
        )

    res = run_bass_kernel_spmd(_get_nc(), in_maps, list(range(GM * GN)))
    _CACHE["last_results"] = res

    out = np.empty((M, N), np.float32)
    for i in range(GM * GN):
        mi, ni = divmod(i, GN)
        out[mi * MC : (mi + 1) * MC, ni * NC : (ni + 1) * NC] = (
            res.results[i]["outT"].T
        )
    return out


# revision 24
# speedup vs baseline: 1.0495x; 1.0316x over previous
"""Trainium2 Bass kernel for nn_AtenMmQuint8: quint8 dense matmul.

    out = ((x - 65) * 0.199) @ ((y - 160) * 0.0215)
    x: [2048, 4096] int32 (quint8 values 0..255)
    y: [4096, 2048] int32 (quint8 values 0..255)
    out: [2048, 2048] fp32

Strategy (v2, fp8 DoubleRow): the correctness gate is rel_err < 2e-2 and
the output is dominated by a large common term (all entries ~ -35.6k +- 2k
in dequant units), so the integer-domain error budget per element is
~100k+ units.  Quantizing both operands to fp8 e4m3 (round-to-nearest)
keeps the total matmul error well inside that budget, which unlocks the
PE's fp8 DoubleRow mode: 256 contraction rows per matmul instead of 128,
i.e. half the bf16 matmul count.

Numerics (verified against the exact int reference on the real inputs):
  - x is re-centered on the host: xc = x - 127 in [-127, 128], so its
    fp8 rounding error (rms 1.79) is much smaller than for x-65 up to
    190 (rms 2.68).  The zero-point shift is corrected EXACTLY:
      out = (xc + 62) @ yd = xc@yd + 62 * colsum(yd)[n]
    The per-n correction is folded into the PSUM->SBUF copy as a
    per-partition bias (the device computes out.T, so n is the
    partition dim).  colsum(yd) is computed exactly on the host in
    int64 (it is part of the affine identity, not an approximation).
  - y ships as fp8(y - 160) directly (rms 2.28).
  - Measured end-to-end: relmax 9.4e-3 vs the 2e-2 gate.

Sharding: 4x2 tensor-parallel grid (4 M-blocks x 2 N-blocks); per-core
block out.T[1024 n, 512 m] = (x_block @ y_block).T.  The device computes
the TRANSPOSED block: stationary operand = y k-tile slice [128k x 128n]
(so out partitions = n and the zero-point bias is per-partition),
moving operand = xT k-tile slice [128k x 512m].

Device kernel (identical SPMD program on all 8 cores):
  - K interleaved across SBUF partitions (k = p*32 + j) exactly as in
    the bf16 kernel; a DoubleRow matmul contracts the (j=2J, j=2J+1)
    pair of k-tiles in one instruction: lhsT/rhs APs are [128, 2, f]
    with the middle dim selecting the pair (sim/ISA-verified layout).
  - 16 double-k-tiles x 8 n-blocks = 128 matmuls; MEASURED warm issue
    rate ~216ns/MM (same N=512 streaming rate as bf16, 2x the MACs)
    -> 27.6us PE floor vs 55.3us for the 256-MM bf16 kernel.
  - The binding resource after that is HBM supply: the three DMA
    queues (2x HWDGE + gpsimd SWDGE) share ~250-280 B/ns aggregate
    while the stream demands 222 B/ns, so the schedule (see comment
    at the scheds) is everything and a few 0.3-2us jitter stalls
    remain.
  - PE prewarm as in v1: throwaway matmuls bridge the gap from the
    framework entry barrier to first-data so the HAM clock ramp is
    done before the real stream starts.  Early PE-idle gaps must stay
    well under ~1us or the un-throttle slips and the early stream
    runs at 1.2GHz (427ns/MM) -- measured, costs multiple us.
  - PSUM: bank nb accumulates n-block nb over all 16 double tiles;
    the last TAIL_J double tiles run nb-major so banks retire one at
    a time: VectorE does (psum * SCALE + bias[n]) in one tensor_scalar
    pass; each 256KB store is split in column halves across the two
    HWDGE rings (loads are SWDGE-only by then), and the final bank
    also splits its scale pass so the kernel-ending chain is
    TS(376) -> 128KB store overlapping the second half.

Measured on the target: 72.6us (bf16 v1 baseline) -> ~47.5us
  = 7.2 entry preamble + ~3.7 warmup/first-data + ~29.5 stream
    (27.6 floor + jitter) + ~1.9 retire/store drain + ~3.7 exit
    (entry/exit are framework-fixed; rel err 9.4e-3 vs 2e-2 gate).
"""

import numpy as np

import concourse.bass as bass  # noqa: F401  (kept for callers/debugging)
import concourse.mybir as mybir
import concourse.tile as tile
from concourse import bacc
from concourse.bass_utils import run_bass_kernel_spmd

X_ZP, Y_ZP = 65, 160
X_SHIFT = 127                # host recenter for x; see _stage()
SCALE = 0.199 * 0.0215

M, K, N = 2048, 4096, 2048
GM, GN = 4, 2                # core grid: 4 M-blocks x 2 N-blocks
MC, NC = M // GM, N // GN    # 512 x 1024 per-core output block
P = 128
KT = K // P                  # 32 k-tiles
KJ = KT // 2                 # 16 double k-tiles (DoubleRow)
NBLK = NC // P               # 8 n-blocks == 8 PSUM banks
NB = 512                     # psum bank free size / matmul moving free dim
TAIL_J = 5                   # trailing double-tiles run nb-major (retire)
N_WARM = 27                  # N=128 warmups after the const-AP burst
N_WARM0 = 10                 # dependency-free N=1 warmups (const-AP weights)

# Load schedules (one FIFO list per DMA ring).  Only SP ("sync") and
# Activation ("scalar") have HWDGE rings; gpsimd DMAs via SWDGE
# (measured 170 B/ns sustained, zero idle).  Pieces:
#   ('x', a, b)  : xb[:, a:b, :]    <- xTr[:, a:b, :]   ((b-a) * 64KB)
#   ('y', a, b)  : yb[:, a:b, :]    <- ysr[:, a:b, :]   ((b-a) * 128KB)
#   ('yh', j, h) : yb[:, j, 512h:512h+512] column half  (64KB)
#   ('bias',)    : bias_sb <- bias dram                 (4KB)
# The tile framework's cross-engine dependencies are byte-range
# bounding boxes, so every matmul of a j-pair gates on ALL FOUR column
# halves of that pair's y (not just its own nb block) -- measured the
# hard way: a schedule that deferred the h1 halves stalled the early
# stream ~1.7us and pushed the HAM un-throttle from 12.5us to 18.3us
# (early PE gaps >~1us delay the clock ramp; <=400ns gaps do not).
# The three queues share ~250-280 B/ns of HBM read bandwidth while the
# stream is running (measured; a queue added does NOT add aggregate),
# so the layout keeps delivery in exact consumption order: the j0-j3
# halves lead both HWDGE rings (h-pairs split across rings so the two
# T0-critical h0 pieces land in parallel), y then alternates ring by
# parity (singles j4-7, pairs j8-27), while SWDGE -- measured at a
# consistent 128-173 B/ns with zero idle -- carries ALL of x in
# deadline order plus the last y pairs, leaving the HWDGE rings idle
# when the tail's output stores need them.
SYNC_SCHED = (
    ("yh", 0, 0), ("yh", 0, 1), ("yh", 2, 0), ("yh", 2, 1),
    ("y", 4, 5), ("y", 6, 7), ("y", 8, 10), ("y", 12, 14),
    ("y", 16, 18), ("y", 20, 22), ("y", 24, 26),
)
SCALAR_SCHED = (
    ("yh", 1, 0), ("yh", 1, 1), ("yh", 3, 0), ("yh", 3, 1),
    ("y", 5, 6), ("y", 7, 8), ("y", 10, 12), ("y", 14, 16),
    ("y", 18, 20), ("y", 22, 24), ("y", 26, 28),
)
# the latest pieces ride SWDGE (idle from ~20us) so the HWDGE rings are
# clear for the tail's output stores.
GPSIMD_SCHED = (
    ("x", 0, 2), ("x", 2, 4), ("x", 4, 6), ("x", 6, 8), ("x", 8, 10),
    ("x", 10, 12), ("x", 12, 14), ("x", 14, 16), ("x", 16, 20),
    ("x", 20, 24), ("x", 24, 28), ("x", 28, 32), ("y", 28, 30),
    ("y", 30, 32), ("bias",),
)


def _check_cover(scheds):
    cover = {("x", j): 0.0 for j in range(KT)}
    cover.update({("y", j): 0.0 for j in range(KT)})
    nbias = 0
    for sched in scheds:
        for item in sched:
            if item[0] == "yh":
                cover[("y", item[1])] += 0.5
            elif item[0] == "bias":
                nbias += 1
            else:
                for j in range(item[1], item[2]):
                    cover[(item[0], j)] += 1
    assert all(v == 1 for v in cover.values()), cover
    assert nbias == 1


_check_cover((SYNC_SCHED, SCALAR_SCHED, GPSIMD_SCHED))


def _emit(tc, xT, ys, bias, outT, tail_j=TAIL_J, n_warm=N_WARM):
    """Emit the per-core device program.

    xT:   [4096, 512]  fp8 DRAM (x block, K-major, host-recentered)
    ys:   [4096, 1024] fp8 DRAM (y block, zero-point subtracted)
    bias: [128, 8]     fp32 DRAM (bias[p, nb] for psum bank nb)
    outT: [1024, 512]  fp32 DRAM (transposed output block)
    """
    nc = tc.nc
    fp32 = mybir.dt.float32
    fp8 = mybir.dt.float8e4

    with (
        tc.tile_pool(name="sb", bufs=1) as sbp,
        tc.tile_pool(name="osb", bufs=NBLK, space="SBUF") as osbp,
        tc.tile_pool(name="ps", bufs=NBLK, space="PSUM") as psp,
    ):
        xb = sbp.tile([P, KT, MC], fp8, name="xb")
        yb = sbp.tile([P, KT, NC], fp8, name="yb")
        bias_sb = sbp.tile([P, NBLK], fp32, name="bias_sb")
        wt = sbp.tile([P, P], fp8, name="wt")
        psum = [psp.tile([P, NB], fp32, tag="ps", name=f"ps_{n}") for n in range(NBLK)]

        # K interleaved across partitions (k = p*KT + j): each partition's
        # j-range is one contiguous DRAM run.
        xTr = xT.rearrange("(p j) m -> p j m", j=KT)
        ysr = ys.rearrange("(p j) n -> p j n", j=KT)

        def issue(eng, sched):
            for item in sched:
                if item[0] == "yh":
                    j, h = item[1], item[2]
                    cs = slice(h * NB, (h + 1) * NB)
                    eng.dma_start(yb[:, j, cs], ysr[:, j, cs])
                elif item[0] == "x":
                    a, b = item[1], item[2]
                    eng.dma_start(xb[:, a:b, :], xTr[:, a:b, :])
                elif item[0] == "y":
                    a, b = item[1], item[2]
                    eng.dma_start(yb[:, a:b, :], ysr[:, a:b, :])
                else:
                    eng.dma_start(bias_sb[:], bias[:])

        nc.vector.memset(wt[:], 0.0)
        issue(nc.sync, SYNC_SCHED)
        issue(nc.scalar, SCALAR_SCHED)
        issue(nc.gpsimd, GPSIMD_SCHED)

        # HAM prewarm: keep the PE continuously busy from the framework
        # entry barrier until the first data lands (see v1 docstring).
        # The first burst uses the framework's const-AP tile ([128,1]
        # fp32, memset by gpsimd BEFORE the entry barrier) as weights, so
        # the PE starts the moment its own preamble ends instead of
        # waiting ~0.5us for the DVE memset of wt to land.
        zap = nc.const_aps.aps[(mybir.dt.float32, 0.0)]
        for _ in range(N_WARM0):
            nc.tensor.matmul(psum[0][:1, :1], zap, zap, start=True, stop=True)
        for _ in range(n_warm):
            nc.tensor.matmul(psum[0][:, :P], wt[:], wt[:], start=True, stop=True)

        def mm(J, nb):
            nc.tensor.matmul(
                psum[nb][:],
                yb[:, 2 * J : 2 * J + 2, nb * P : (nb + 1) * P],
                xb[:, 2 * J : 2 * J + 2, :],
                start=(J == 0),
                stop=(J == KJ - 1),
                perf_mode=mybir.MatmulPerfMode.DoubleRow,
            )

        # J-outer: touch every psum bank each double tile so the PE stream
        # stays dense while loads race ahead.
        for J in range(KJ - tail_j):
            for nb in range(NBLK):
                mm(J, nb)
        # nb-major tail: each PSUM bank finishes its K accumulation alone,
        # so its scale+bias copy and 256KB store overlap the remaining
        # matmuls of the other banks (stores alternate rings: 2MB total
        # needs both).
        for nb in range(NBLK):
            for J in range(KJ - tail_j, KJ):
                mm(J, nb)
            osb = osbp.tile([P, NB], fp32, tag="osb", name=f"osb_{nb}")
            rows = slice(nb * P, (nb + 1) * P)
            h = NB // 2
            if nb < NBLK - 1:
                nc.vector.tensor_scalar(
                    osb[:], psum[nb][:], SCALE, bias_sb[:, nb : nb + 1],
                    mybir.AluOpType.mult, mybir.AluOpType.add,
                )
                # store in column halves on both rings: halves each
                # store's drain (the load schedules are long done by the
                # time stores begin).
                nc.sync.dma_start(outT[rows, 0:h], osb[:, 0:h])
                nc.scalar.dma_start(outT[rows, h:NB], osb[:, h:NB])
            else:
                # last bank is the kernel-ending chain: retire in column
                # halves so the first 128KB store overlaps the second
                # scale pass.  (Quarters were tried: the TS fixed cost
                # ~215ns makes 4 passes a LONGER serial chain than 2.
                # An ACT-engine half was also tried for engine-parallel
                # retirement: ~2x slower than DVE per element, net wash.)
                for k in range(2):
                    cs = slice(k * h, (k + 1) * h)
                    nc.vector.tensor_scalar(
                        osb[:, cs], psum[nb][:, cs], SCALE,
                        bias_sb[:, nb : nb + 1],
                        mybir.AluOpType.mult, mybir.AluOpType.add,
                    )
                    eng = nc.sync if k % 2 == 0 else nc.scalar
                    eng.dma_start(outT[rows, cs], osb[:, cs])


def _build_nc(**emit_kw):
    nc = bacc.Bacc("TRN2", target_bir_lowering=False, debug=False)
    fp8 = mybir.dt.float8e4
    xT = nc.declare_dram_parameter("xT", [K, MC], fp8, isOutput=False)
    ys = nc.declare_dram_parameter("ys", [K, NC], fp8, isOutput=False)
    bias = nc.declare_dram_parameter("bias", [P, NBLK], mybir.dt.float32,
                                     isOutput=False)
    outT = nc.declare_dram_parameter("outT", [NC, MC], mybir.dt.float32,
                                     isOutput=True)
    with tile.TileContext(nc) as tc:
        _emit(tc, xT[:], ys[:], bias[:], outT[:], **emit_kw)
    nc.compile()
    return nc


_CACHE = {}


def _get_nc():
    if "nc" not in _CACHE:
        _CACHE["nc"] = _build_nc()
    return _CACHE["nc"]


def _stage(x, y):
    """Host staging: fp8 operands + exact zero-point-shift bias."""
    fp8_np = mybir.dt.np(mybir.dt.float8e4)
    # x recentered to [-127, 128]; correction is exact via colsum(yd).
    xc8 = (x.astype(np.float32) - np.float32(X_SHIFT)).astype(fp8_np)
    yd8 = (y.astype(np.float32) - np.float32(Y_ZP)).astype(fp8_np)
    xT8 = np.ascontiguousarray(xc8.T)  # [K, M] fp8
    # out = xc@yd + (X_SHIFT - X_ZP) * colsum(yd);  62 = 127 - 65
    colsum = (y.astype(np.int64) - Y_ZP).sum(axis=0)  # [N] exact
    biasvec = (float(X_SHIFT - X_ZP) * colsum.astype(np.float64) * SCALE
               ).astype(np.float32)
    return xT8, yd8, biasvec


def kernel(x, y):
    x = np.asarray(x)
    y = np.asarray(y)
    assert x.shape == (M, K) and y.shape == (K, N)
    xT8, yd8, biasvec = _stage(x, y)

    in_maps = []
    for i in range(GM * GN):
        mi, ni = divmod(i, GN)
        bv = biasvec[ni * NC : (ni + 1) * NC].reshape(NBLK, P).T  # [P, NBLK]
        in_maps.append(
            {
                "xT": np.ascontiguousarray(xT8[:, mi * MC : (mi + 1) * MC]),
                "ys": np.ascontiguousarray(yd8[:, ni * NC : (ni + 1) * NC]),
                "bias": np.ascontiguousarray(bv),
            }
        )

    res = run_bass_kernel_spmd(_get_nc(), in_maps, list(range(GM * GN)))
    _CACHE["last_results"] = res

    out = np.empty((M, N), np.float32)
    for i in range(GM * GN):
        mi, ni = divmod(i, GN)
        out[mi * MC : (mi + 1) * MC, ni * NC : (ni + 1) * NC] = (
            res.results[i]["outT"].T
        )
    return out
